# revision 17
# baseline (speedup 1.0000x reference)
"""MiniSTU Trainium2 kernel (8 NeuronCores, Bass/Tile).

Math: the reference's FFT convolution + einsum collapses to
    y[b,l,o] = sum_g sum_{t<=l} phi_eff_g[l-t] * (x[b,t] @ M_g)[o]
over g in the 48 (filter k, sign) pairs, where phi_eff carries the
(-1)^s alternation for the minus branch (the two sgn factors in the
reference combine to (-1)^(l-t), i.e. an alternating filter).

Device algorithm per core (6 pairs per core, filter-dim sharding):
  stage 1: Z_g[t, (b,o)] = xT_tile.T @ M_g       (PE, f32r)
  stage 2: y[c] += Toeplitz(phi_eff_g)[c-cp].T @ Z_g[cp]   (PE, f32r)
Toeplitz blocks are expanded on host from phi. The 8 per-core partial
outputs are summed on host (the gather for this sharding).
"""

import numpy as np

import concourse.bass as bass
import concourse.tile as tile
from concourse import mybir
from concourse.bass_utils import run_bass_kernel_spmd
from concourse.vector_clock import ScopedClock

L = 2048
K = 24
I = 256
O = 256
B = 2
TS = 128          # tile size along sequence
CT = L // TS      # 16 sequence tiles
NP = 6            # (k, sign) pairs per core
N_CORES = 8
BO = B * O        # 512 fused (b, o) columns
F32 = mybir.dt.float32
F32R = mybir.dt.float32r


# ---------------------------------------------------------------------------
# Workarounds for this container's walrus: it rejects any instruction that
# carries more than one sync-wait command.
# ---------------------------------------------------------------------------

def _prune_init_barrier(nc):
    """Drop the Bass-init all-engine EVSEM barrier and the unused const
    memsets from the 'main' bb (~3us of EVSEM latency before any work).
    Register init is per-engine; Tile emits its own sems for every
    cross-engine dependency, so the startup barrier guards nothing here."""
    for f in nc.m.functions:
        for blk in f.blocks:
            if blk.name != "main":
                continue
            keep = []
            for inst in blk.instructions:
                nm = type(inst).__name__
                if nm in ("InstMemset", "InstDrain", "InstEventSemaphore"):
                    continue
                keep.append(inst)
            blk.instructions = keep


def _split_sync_waits(nc, max_waits=1):
    """Hoist extra sem-waits onto same-engine NOPs inserted right before the
    offending instruction; queue order keeps the semantics identical."""
    for f in nc.m.functions:
        for blk in f.blocks:
            insts = list(blk.instructions)
            out = []
            changed = False
            for inst in insts:
                si = getattr(inst, "sync_info", None)
                waits = list(si.on_wait) if si is not None else []
                if len(waits) > max_waits:
                    changed = True
                    extra, keep = waits[:-max_waits], waits[-max_waits:]
                    for j in range(0, len(extra), max_waits):
                        nop = mybir.InstNoOp(
                            name=nc.get_next_instruction_name(), ins=[], outs=[]
                        )
                        nop.engine = inst.engine
                        nop.sync_info = mybir.SyncInfo(
                            on_wait=extra[j : j + max_waits], on_update=[]
                        )
                        out.append(nop)
                    inst.sync_info = mybir.SyncInfo(
                        on_wait=keep, on_update=list(si.on_update)
                    )
                out.append(inst)
            if changed:
                blk.instructions = out


class _TC(tile.TileContext):
    """TileContext whose tail drain spreads its waits over 1-wait NOPs."""

    def _drain_and_barrier(self, tick_clock, wait_clock):
        nc = self.nc
        nop_inst = nc.sync.nop()
        wait_clock.add_sem_waits(
            nop_inst.ins, ScopedClock({None: tick_clock.global_clock})
        )
        si = nop_inst.ins.sync_info
        if si is not None and len(si.on_wait) > 1:
            waits = list(si.on_wait)
            nop_inst.ins.sync_info = mybir.SyncInfo(
                on_wait=waits[:1], on_update=list(si.on_update)
            )
            for w in waits[1:]:
                extra = nc.sync.nop().ins
                extra.sync_info = mybir.SyncInfo(on_wait=[w], on_update=[])
        nc.sync.drain()
        # No tail barriers / sem clearing: nothing runs after this kernel,
        # and all output DMAs are issued (and drained) on the SP queue.
        assert self.sems is not None
        popped = nc._tile_sem_poison_stack.pop()
        assert popped is self._sem_poison


# ---------------------------------------------------------------------------
# Device program (identical on all 8 cores; per-core data differs)
# ---------------------------------------------------------------------------

def _build_nc():
    nc = bass.Bass("TRN2", target_bir_lowering=False, debug=False,
                   num_devices=N_CORES)
    # x batched per sequence tile: [cp, i, (b, ic, t)]
    xT_d = nc.dram_tensor("xT", [CT, TS, B * 2 * TS], F32R, kind="ExternalInput")
    # M fused per pair-pair: [pp, ic, i, (p0 o | p1 o)]
    m_d = nc.dram_tensor("m", [NP // 2, 2, TS, 2 * O], F32R, kind="ExternalInput")
    # Toeplitz blocks batched per diagonal: [d, t, (p, l)]
    tb_d = nc.dram_tensor("tb", [CT, TS, NP * TS], F32R, kind="ExternalInput")
    yp_d = nc.dram_tensor("yp", [CT, TS, BO], F32, kind="ExternalOutput")

    with _TC(nc) as tc:
        with (
            tc.tile_pool(name="const", bufs=1) as cpool,
            tc.tile_pool(name="ys", bufs=1) as ypool,
            tc.tile_pool(name="z", bufs=18) as zpool,
            tc.tile_pool(name="ps1", bufs=4, space="PSUM") as ps1,
            tc.tile_pool(name="ps2", bufs=4, space="PSUM") as ps2,
        ):
            ms = [[cpool.tile([TS, 2 * O], F32R, tag=f"m{pp}{ic}", name=f"m{pp}{ic}")
                   for ic in range(2)] for pp in range(NP // 2)]
            xs = [cpool.tile([TS, B * 2 * TS], F32R, tag=f"x{cp}", name=f"x{cp}")
                  for cp in range(CT)]
            tbs = [cpool.tile([TS, NP * TS], F32R, tag=f"t{d}", name=f"t{d}")
                   for d in range(CT)]
            # strict consumption order; first stage-1 group only needs
            # ms[0][*] + x0, so those go at the head of both queues
            nc.gpsimd.dma_start(ms[0][0][:], m_d[0, 0])
            nc.gpsimd.dma_start(ms[0][1][:], m_d[0, 1])
            nc.gpsimd.dma_start(xs[0][:], xT_d[0])
            for pp in range(1, NP // 2):
                nc.gpsimd.dma_start(ms[pp][0][:], m_d[pp, 0])
                nc.sync.dma_start(ms[pp][1][:], m_d[pp, 1])
            nc.gpsimd.dma_start(xs[1][:], xT_d[1])
            for s in range(CT):
                if s >= 2:
                    nc.gpsimd.dma_start(xs[s][:], xT_d[s])
                nc.sync.dma_start(tbs[s][:], tb_d[s])


            y_sb = [ypool.tile([TS, BO], F32, tag=f"y{c}", name=f"ysb{c}") for c in range(CT)]

            for q in range(0, CT, 2):
                # stage 1 for cp = q and q+1: Z[t, (b0 o | b1 o)] per pair
                zall = {}
                for cp in (q, q + 1):
                    zts = [zpool.tile([TS, BO], F32R, tag="z", name=f"z{cp}_{p}")
                           for p in range(NP)]
                    for pp in range(NP // 2):
                        pss = []
                        for b in range(B):
                            ps = ps1.tile([TS, BO], F32, tag="s1")
                            for ic in range(2):
                                nc.tensor.matmul(
                                    ps[:],
                                    xs[cp][:, (b * 2 + ic) * TS:(b * 2 + ic + 1) * TS],
                                    ms[pp][ic][:],
                                    start=(ic == 0),
                                    stop=(ic == 1),
                                )
                            pss.append(ps)
                        for h in range(2):
                            z = zts[2 * pp + h]
                            for b in range(B):
                                nc.vector.tensor_copy(
                                    z[:, b * O:(b + 1) * O],
                                    pss[b][:, h * O:(h + 1) * O],
                                )
                    zall[cp] = zts
                # stage 2: contributions of both tiles to every c >= q,
                # fused into one PSUM accumulation group per c
                for c in range(q, CT):
                    cps = [cp for cp in (q, q + 1) if cp <= c]
                    yps = ps2.tile([TS, BO], F32, tag="s2")
                    n_mm = len(cps) * NP
                    i_mm = 0
                    for cp in cps:
                        for p in range(NP):
                            nc.tensor.matmul(
                                yps[:],
                                tbs[c - cp][:, p * TS:(p + 1) * TS],
                                zall[cp][p][:],
                                start=(i_mm == 0),
                                stop=(i_mm == n_mm - 1),
                            )
                            i_mm += 1
                    if q == 0:
                        nc.vector.tensor_copy(y_sb[c][:], yps[:])
                    else:
                        nc.vector.tensor_add(y_sb[c][:], y_sb[c][:], yps[:])
                    if c <= q + 1:  # y_sb[c] complete once its own tile passed
                        nc.sync.dma_start(yp_d[c], y_sb[c][:])

    _prune_init_barrier(nc)
    _split_sync_waits(nc)
    return nc


# ---------------------------------------------------------------------------
# Host side: input staging, sharding, gather
# ---------------------------------------------------------------------------

def _build_toeplitz(phi_eff):
    """tb[d, t, l] = phi_eff[d*TS + l - t] (0 where the index is negative)."""
    pad = np.zeros(L + TS - 1, np.float32)
    pad[TS - 1:] = phi_eff
    d = np.arange(CT)[:, None, None]
    t = np.arange(TS)[None, :, None]
    l = np.arange(TS)[None, None, :]
    return pad[d * TS + l - t + TS - 1]


_last_in_maps = None  # stashed for external profiling harnesses


def _prepare(x, phi, M_phi_plus, M_phi_minus):
    """Host prep: build per-core in_maps (no device execution)."""
    x = np.asarray(x, np.float32)
    phi = np.asarray(phi, np.float32)
    Mp = np.asarray(M_phi_plus, np.float32)
    Mm = np.asarray(M_phi_minus, np.float32)

    # [cp, i, (b, ic, t)]: per-sequence-tile chunks of x^T, one DMA per cp
    xT = np.ascontiguousarray(
        x.reshape(B, CT, TS, 2, TS).transpose(1, 4, 0, 3, 2)
    ).reshape(CT, TS, B * 2 * TS)
    sgn = ((-1.0) ** np.arange(L)).astype(np.float32)

    m_all = np.empty((2 * K, 2, TS, O), np.float32)
    tb_all = np.empty((2 * K, CT, TS, TS), np.float32)
    for g in range(2 * K):
        k, s = g // 2, g % 2
        m_all[g] = (Mm if s else Mp)[k].reshape(2, TS, O)
        phi_eff = phi[:, k] * (sgn if s else 1.0)
        tb_all[g] = _build_toeplitz(phi_eff)
    # fuse pair-pairs into columns: [pp, ic, i, (g0 o | g1 o)] per core
    m_fused = np.concatenate(
        [m_all[0::2], m_all[1::2]], axis=3
    )  # [24, 2, TS, 2*O] where row j holds (g=2j | g=2j+1)

    nc = _build_nc()
    in_maps = []
    for core in range(N_CORES):
        gs = slice(core * NP, (core + 1) * NP)
        pps = slice(core * (NP // 2), (core + 1) * (NP // 2))
        # [d, t, (p, l)] diagonal-major Toeplitz blocks for this core
        tb_core = np.ascontiguousarray(
            tb_all[gs].transpose(1, 2, 0, 3)
        ).reshape(CT, TS, NP * TS)
        in_maps.append({
            "xT": xT,
            "m": np.ascontiguousarray(m_fused[pps]),
            "tb": tb_core,
        })
    _last_in_maps = in_maps
    return nc, in_maps


def _gather(results):
    y = np.zeros((CT, TS, B, O), np.float64)
    for core in range(N_CORES):
        y += results[core]["yp"].reshape(CT, TS, B, O)
    return np.ascontiguousarray(
        y.transpose(2, 0, 1, 3).reshape(B, L, O)
    ).astype(np.float32)


def kernel(x, phi, M_phi_plus, M_phi_minus):
    nc, in_maps = _prepare(x, phi, M_phi_plus, M_phi_minus)
    res = run_bass_kernel_spmd(nc, in_maps, list(range(N_CORES)))
    return _gather(res.results)


# revision 18
# speedup vs baseline: 1.0035x; 1.0035x over previous
"""MiniSTU Trainium2 kernel (8 NeuronCores, Bass/Tile).

Math: the reference's FFT convolution + einsum collapses to
    y[b,l,o] = sum_g sum_{t<=l} phi_eff_g[l-t] * (x[b,t] @ M_g)[o]
over g in the 48 (filter k, sign) pairs, where phi_eff carries the
(-1)^s alternation for the minus branch (the two sgn factors in the
reference combine to (-1)^(l-t), i.e. an alternating filter).

Device algorithm per core (6 pairs per core, filter-dim sharding):
  stage 1: Z_g[t, (b,o)] = xT_tile.T @ M_g       (PE, f32r)
  stage 2: y[c] += Toeplitz(phi_eff_g)[c-cp].T @ Z_g[cp]   (PE, f32r)
Toeplitz blocks are expanded on host from phi. The 8 per-core partial
outputs are summed on host (the gather for this sharding).
"""

import numpy as np

import concourse.bass as bass
import concourse.tile as tile
from concourse import mybir
from concourse.bass_utils import run_bass_kernel_spmd
from concourse.vector_clock import ScopedClock

L = 2048
K = 24
I = 256
O = 256
B = 2
TS = 128          # tile size along sequence
CT = L // TS      # 16 sequence tiles
NP = 6            # (k, sign) pairs per core
N_CORES = 8
BO = B * O        # 512 fused (b, o) columns
F32 = mybir.dt.float32
F32R = mybir.dt.float32r


# ---------------------------------------------------------------------------
# Workarounds for this container's walrus: it rejects any instruction that
# carries more than one sync-wait command.
# ---------------------------------------------------------------------------

def _prune_init_barrier(nc):
    """Drop the Bass-init all-engine EVSEM barrier and the unused const
    memsets from the 'main' bb (~3us of EVSEM latency before any work).
    Register init is per-engine; Tile emits its own sems for every
    cross-engine dependency, so the startup barrier guards nothing here."""
    for f in nc.m.functions:
        for blk in f.blocks:
            if blk.name != "main":
                continue
            keep = []
            for inst in blk.instructions:
                nm = type(inst).__name__
                if nm in ("InstMemset", "InstDrain", "InstEventSemaphore"):
                    continue
                keep.append(inst)
            blk.instructions = keep


def _split_sync_waits(nc, max_waits=1):
    """Hoist extra sem-waits onto same-engine NOPs inserted right before the
    offending instruction; queue order keeps the semantics identical."""
    for f in nc.m.functions:
        for blk in f.blocks:
            insts = list(blk.instructions)
            out = []
            changed = False
            for inst in insts:
                si = getattr(inst, "sync_info", None)
                waits = list(si.on_wait) if si is not None else []
                if len(waits) > max_waits:
                    changed = True
                    extra, keep = waits[:-max_waits], waits[-max_waits:]
                    for j in range(0, len(extra), max_waits):
                        nop = mybir.InstNoOp(
                            name=nc.get_next_instruction_name(), ins=[], outs=[]
                        )
                        nop.engine = inst.engine
                        nop.sync_info = mybir.SyncInfo(
                            on_wait=extra[j : j + max_waits], on_update=[]
                        )
                        out.append(nop)
                    inst.sync_info = mybir.SyncInfo(
                        on_wait=keep, on_update=list(si.on_update)
                    )
                out.append(inst)
            if changed:
                blk.instructions = out


class _TC(tile.TileContext):
    """TileContext whose tail drain spreads its waits over 1-wait NOPs."""

    def _drain_and_barrier(self, tick_clock, wait_clock):
        nc = self.nc
        nop_inst = nc.sync.nop()
        wait_clock.add_sem_waits(
            nop_inst.ins, ScopedClock({None: tick_clock.global_clock})
        )
        si = nop_inst.ins.sync_info
        if si is not None and len(si.on_wait) > 1:
            waits = list(si.on_wait)
            nop_inst.ins.sync_info = mybir.SyncInfo(
                on_wait=waits[:1], on_update=list(si.on_update)
            )
            for w in waits[1:]:
                extra = nc.sync.nop().ins
                extra.sync_info = mybir.SyncInfo(on_wait=[w], on_update=[])
        nc.sync.drain()
        # No tail barriers / sem clearing: nothing runs after this kernel,
        # and all output DMAs are issued (and drained) on the SP queue.
        assert self.sems is not None
        popped = nc._tile_sem_poison_stack.pop()
        assert popped is self._sem_poison


# ---------------------------------------------------------------------------
# Device program (identical on all 8 cores; per-core data differs)
# ---------------------------------------------------------------------------

def _build_nc():
    nc = bass.Bass("TRN2", target_bir_lowering=False, debug=False,
                   num_devices=N_CORES)
    # x batched per sequence tile: [cp, i, (b, ic, t)]
    xT_d = nc.dram_tensor("xT", [CT, TS, B * 2 * TS], F32R, kind="ExternalInput")
    # M fused per pair-pair: [pp, ic, i, (p0 o | p1 o)]
    m_d = nc.dram_tensor("m", [NP // 2, 2, TS, 2 * O], F32R, kind="ExternalInput")
    # Toeplitz blocks batched per diagonal: [d, t, (p, l)]
    tb_d = nc.dram_tensor("tb", [CT, TS, NP * TS], F32R, kind="ExternalInput")
    yp_d = nc.dram_tensor("yp", [CT, TS, BO], F32, kind="ExternalOutput")

    with _TC(nc) as tc:
        with (
            tc.tile_pool(name="const", bufs=1) as cpool,
            tc.tile_pool(name="ys", bufs=1) as ypool,
            tc.tile_pool(name="z", bufs=18) as zpool,
            tc.tile_pool(name="ps1", bufs=4, space="PSUM") as ps1,
            tc.tile_pool(name="ps2", bufs=4, space="PSUM") as ps2,
        ):
            ms = [[cpool.tile([TS, 2 * O], F32R, tag=f"m{pp}{ic}", name=f"m{pp}{ic}")
                   for ic in range(2)] for pp in range(NP // 2)]
            xs = [cpool.tile([TS, B * 2 * TS], F32R, tag=f"x{cp}", name=f"x{cp}")
                  for cp in range(CT)]
            tbs = [cpool.tile([TS, NP * TS], F32R, tag=f"t{d}", name=f"t{d}")
                   for d in range(CT)]
            # strict consumption order; first stage-1 group only needs
            # ms[0][*] + x0, so those go at the head of both queues
            nc.sync.dma_start(ms[0][0][:], m_d[0, 0])
            nc.gpsimd.dma_start(ms[0][1][:], m_d[0, 1])
            nc.sync.dma_start(xs[0][:], xT_d[0])
            for pp in range(1, NP // 2):
                nc.gpsimd.dma_start(ms[pp][0][:], m_d[pp, 0])
                nc.sync.dma_start(ms[pp][1][:], m_d[pp, 1])
            nc.gpsimd.dma_start(xs[1][:], xT_d[1])
            for s in range(CT):
                if s >= 2:
                    nc.gpsimd.dma_start(xs[s][:], xT_d[s])
                nc.sync.dma_start(tbs[s][:], tb_d[s])


            y_sb = [ypool.tile([TS, BO], F32, tag=f"y{c}", name=f"ysb{c}") for c in range(CT)]

            for q in range(0, CT, 2):
                # stage 1 for cp = q and q+1: Z[t, (b0 o | b1 o)] per pair
                zall = {}
                for cp in (q, q + 1):
                    zts = [zpool.tile([TS, BO], F32R, tag="z", name=f"z{cp}_{p}")
                           for p in range(NP)]
                    for pp in range(NP // 2):
                        pss = []
                        for b in range(B):
                            ps = ps1.tile([TS, BO], F32, tag="s1")
                            for ic in range(2):
                                nc.tensor.matmul(
                                    ps[:],
                                    xs[cp][:, (b * 2 + ic) * TS:(b * 2 + ic + 1) * TS],
                                    ms[pp][ic][:],
                                    start=(ic == 0),
                                    stop=(ic == 1),
                                )
                            pss.append(ps)
                        for h in range(2):
                            z = zts[2 * pp + h]
                            for b in range(B):
                                nc.vector.tensor_copy(
                                    z[:, b * O:(b + 1) * O],
                                    pss[b][:, h * O:(h + 1) * O],
                                )
                    zall[cp] = zts
                # stage 2: contributions of both tiles to every c >= q,
                # fused into one PSUM accumulation group per c
                for c in range(q, CT):
                    cps = [cp for cp in (q, q + 1) if cp <= c]
                    yps = ps2.tile([TS, BO], F32, tag="s2")
                    n_mm = len(cps) * NP
                    i_mm = 0
                    for cp in cps:
                        for p in range(NP):
                            nc.tensor.matmul(
                                yps[:],
                                tbs[c - cp][:, p * TS:(p + 1) * TS],
                                zall[cp][p][:],
                                start=(i_mm == 0),
                                stop=(i_mm == n_mm - 1),
                            )
                            i_mm += 1
                    if q == 0:
                        nc.vector.tensor_copy(y_sb[c][:], yps[:])
                    else:
                        nc.vector.tensor_add(y_sb[c][:], y_sb[c][:], yps[:])
                    if c <= q + 1:  # y_sb[c] complete once its own tile passed
                        nc.sync.dma_start(yp_d[c], y_sb[c][:])

    _prune_init_barrier(nc)
    _split_sync_waits(nc)
    return nc


# ---------------------------------------------------------------------------
# Host side: input staging, sharding, gather
# ---------------------------------------------------------------------------

def _build_toeplitz(phi_eff):
    """tb[d, t, l] = phi_eff[d*TS + l - t] (0 where the index is negative)."""
    pad = np.zeros(L + TS - 1, np.float32)
    pad[TS - 1:] = phi_eff
    d = np.arange(CT)[:, None, None]
    t = np.arange(TS)[None, :, None]
    l = np.arange(TS)[None, None, :]
    return pad[d * TS + l - t + TS - 1]


_last_in_maps = None  # stashed for external profiling harnesses


def _prepare(x, phi, M_phi_plus, M_phi_minus):
    """Host prep: build per-core in_maps (no device execution)."""
    x = np.asarray(x, np.float32)
    phi = np.asarray(phi, np.float32)
    Mp = np.asarray(M_phi_plus, np.float32)
    Mm = np.asarray(M_phi_minus, np.float32)

    # [cp, i, (b, ic, t)]: per-sequence-tile chunks of x^T, one DMA per cp
    xT = np.ascontiguousarray(
        x.reshape(B, CT, TS, 2, TS).transpose(1, 4, 0, 3, 2)
    ).reshape(CT, TS, B * 2 * TS)
    sgn = ((-1.0) ** np.arange(L)).astype(np.float32)

    m_all = np.empty((2 * K, 2, TS, O), np.float32)
    tb_all = np.empty((2 * K, CT, TS, TS), np.float32)
    for g in range(2 * K):
        k, s = g // 2, g % 2
        m_all[g] = (Mm if s else Mp)[k].reshape(2, TS, O)
        phi_eff = phi[:, k] * (sgn if s else 1.0)
        tb_all[g] = _build_toeplitz(phi_eff)
    # fuse pair-pairs into columns: [pp, ic, i, (g0 o | g1 o)] per core
    m_fused = np.concatenate(
        [m_all[0::2], m_all[1::2]], axis=3
    )  # [24, 2, TS, 2*O] where row j holds (g=2j | g=2j+1)

    nc = _build_nc()
    in_maps = []
    for core in range(N_CORES):
        gs = slice(core * NP, (core + 1) * NP)
        pps = slice(core * (NP // 2), (core + 1) * (NP // 2))
        # [d, t, (p, l)] diagonal-major Toeplitz blocks for this core
        tb_core = np.ascontiguousarray(
            tb_all[gs].transpose(1, 2, 0, 3)
        ).reshape(CT, TS, NP * TS)
        in_maps.append({
            "xT": xT,
            "m": np.ascontiguousarray(m_fused[pps]),
            "tb": tb_core,
        })
    _last_in_maps = in_maps
    return nc, in_maps


def _gather(results):
    y = np.zeros((CT, TS, B, O), np.float64)
    for core in range(N_CORES):
        y += results[core]["yp"].reshape(CT, TS, B, O)
    return np.ascontiguousarray(
        y.transpose(2, 0, 1, 3).reshape(B, L, O)
    ).astype(np.float32)


def kernel(x, phi, M_phi_plus, M_phi_minus):
    nc, in_maps = _prepare(x, phi, M_phi_plus, M_phi_minus)
    res = run_bass_kernel_spmd(nc, in_maps, list(range(N_CORES)))
    return _gather(res.results)


# revision 20
# speedup vs baseline: 1.0064x; 1.0029x over previous
"""MiniSTU Trainium2 kernel (8 NeuronCores, Bass/Tile).

Math: the reference's FFT convolution + einsum collapses to
    y[b,l,o] = sum_g sum_{t<=l} phi_eff_g[l-t] * (x[b,t] @ M_g)[o]
over g in the 48 (filter k, sign) pairs, where phi_eff carries the
(-1)^s alternation for the minus branch (the two sgn factors in the
reference combine to (-1)^(l-t), i.e. an alternating filter).

Device algorithm per core (6 pairs per core, filter-dim sharding):
  stage 1: Z_g[t, (b,o)] = xT_tile.T @ M_g       (PE, f32r)
  stage 2: y[c] += Toeplitz(phi_eff_g)[c-cp].T @ Z_g[cp]   (PE, f32r)
Toeplitz blocks are expanded on host from phi. The 8 per-core partial
outputs are summed on host (the gather for this sharding).
"""

import numpy as np

import concourse.bass as bass
import concourse.tile as tile
from concourse import mybir
from concourse.bass_utils import run_bass_kernel_spmd
from concourse.vector_clock import ScopedClock

L = 2048
K = 24
I = 256
O = 256
B = 2
TS = 128          # tile size along sequence
CT = L // TS      # 16 sequence tiles
NP = 6            # (k, sign) pairs per core
N_CORES = 8
BO = B * O        # 512 fused (b, o) columns
F32 = mybir.dt.float32
F32R = mybir.dt.float32r


# ---------------------------------------------------------------------------
# Workarounds for this container's walrus: it rejects any instruction that
# carries more than one sync-wait command.
# ---------------------------------------------------------------------------

def _prune_init_barrier(nc):
    """Drop the Bass-init all-engine EVSEM barrier and the unused const
    memsets from the 'main' bb (~3us of EVSEM latency before any work).
    Register init is per-engine; Tile emits its own sems for every
    cross-engine dependency, so the startup barrier guards nothing here."""
    for f in nc.m.functions:
        for blk in f.blocks:
            if blk.name != "main":
                continue
            keep = []
            for inst in blk.instructions:
                nm = type(inst).__name__
                if nm in ("InstMemset", "InstDrain", "InstEventSemaphore"):
                    continue
                keep.append(inst)
            blk.instructions = keep


def _split_sync_waits(nc, max_waits=1):
    """Hoist extra sem-waits onto same-engine NOPs inserted right before the
    offending instruction; queue order keeps the semantics identical."""
    for f in nc.m.functions:
        for blk in f.blocks:
            insts = list(blk.instructions)
            out = []
            changed = False
            for inst in insts:
                si = getattr(inst, "sync_info", None)
                waits = list(si.on_wait) if si is not None else []
                if len(waits) > max_waits:
                    changed = True
                    extra, keep = waits[:-max_waits], waits[-max_waits:]
                    for j in range(0, len(extra), max_waits):
                        nop = mybir.InstNoOp(
                            name=nc.get_next_instruction_name(), ins=[], outs=[]
                        )
                        nop.engine = inst.engine
                        nop.sync_info = mybir.SyncInfo(
                            on_wait=extra[j : j + max_waits], on_update=[]
                        )
                        out.append(nop)
                    inst.sync_info = mybir.SyncInfo(
                        on_wait=keep, on_update=list(si.on_update)
                    )
                out.append(inst)
            if changed:
                blk.instructions = out


class _TC(tile.TileContext):
    """TileContext whose tail drain spreads its waits over 1-wait NOPs."""

    def _drain_and_barrier(self, tick_clock, wait_clock):
        nc = self.nc
        nop_inst = nc.sync.nop()
        wait_clock.add_sem_waits(
            nop_inst.ins, ScopedClock({None: tick_clock.global_clock})
        )
        si = nop_inst.ins.sync_info
        if si is not None and len(si.on_wait) > 1:
            waits = list(si.on_wait)
            nop_inst.ins.sync_info = mybir.SyncInfo(
                on_wait=waits[:1], on_update=list(si.on_update)
            )
            for w in waits[1:]:
                extra = nc.sync.nop().ins
                extra.sync_info = mybir.SyncInfo(on_wait=[w], on_update=[])
        nc.sync.drain()
        # No tail barriers / sem clearing: nothing runs after this kernel,
        # and all output DMAs are issued (and drained) on the SP queue.
        assert self.sems is not None
        popped = nc._tile_sem_poison_stack.pop()
        assert popped is self._sem_poison


# ---------------------------------------------------------------------------
# Device program (identical on all 8 cores; per-core data differs)
# ---------------------------------------------------------------------------

def _build_nc():
    nc = bass.Bass("TRN2", target_bir_lowering=False, debug=False,
                   num_devices=N_CORES)
    # x batched per sequence tile: [cp, i, (b, ic, t)]
    xT_d = nc.dram_tensor("xT", [CT, TS, B * 2 * TS], F32R, kind="ExternalInput")
    # M fused per pair-pair: [pp, ic, i, (p0 o | p1 o)]
    m_d = nc.dram_tensor("m", [NP // 2, 2, TS, 2 * O], F32R, kind="ExternalInput")
    # Toeplitz blocks batched per diagonal: [d, t, (p, l)]
    tb_d = nc.dram_tensor("tb", [CT, TS, NP * TS], F32R, kind="ExternalInput")
    yp_d = nc.dram_tensor("yp", [CT, TS, BO], F32, kind="ExternalOutput")

    with _TC(nc) as tc:
        with (
            tc.tile_pool(name="const", bufs=1) as cpool,
            tc.tile_pool(name="ys", bufs=1) as ypool,
            tc.tile_pool(name="z", bufs=18) as zpool,
            tc.tile_pool(name="ps1", bufs=4, space="PSUM") as ps1,
            tc.tile_pool(name="ps2", bufs=4, space="PSUM") as ps2,
        ):
            ms = [[cpool.tile([TS, 2 * O], F32R, tag=f"m{pp}{ic}", name=f"m{pp}{ic}")
                   for ic in range(2)] for pp in range(NP // 2)]
            xs = [cpool.tile([TS, B * 2 * TS], F32R, tag=f"x{cp}", name=f"x{cp}")
                  for cp in range(CT)]
            tbs = [cpool.tile([TS, NP * TS], F32R, tag=f"t{d}", name=f"t{d}")
                   for d in range(CT)]
            # strict consumption order; the critical head transfers
            # (ms[0][*], x0) are spread over four different DMA queues so
            # the first stage-1 group is not gated on one ring's bandwidth
            nc.sync.dma_start(ms[0][0][:], m_d[0, 0])
            nc.gpsimd.dma_start(ms[0][1][:], m_d[0, 1])
            nc.scalar.dma_start(xs[0][:], xT_d[0])
            nc.sync.dma_start(ms[1][0][:], m_d[1, 0])
            nc.gpsimd.dma_start(ms[1][1][:], m_d[1, 1])
            nc.scalar.dma_start(ms[2][0][:], m_d[2, 0])
            nc.sync.dma_start(ms[2][1][:], m_d[2, 1])
            nc.gpsimd.dma_start(xs[1][:], xT_d[1])
            nc.sync.dma_start(tbs[0][:], tb_d[0])
            for s in range(1, CT):
                if s + 1 < CT:
                    nc.gpsimd.dma_start(xs[s + 1][:], xT_d[s + 1])
                nc.sync.dma_start(tbs[s][:], tb_d[s])


            y_sb = [ypool.tile([TS, BO], F32, tag=f"y{c}", name=f"ysb{c}") for c in range(CT)]

            for q in range(0, CT, 2):
                # stage 1 for cp = q and q+1: Z[t, (b0 o | b1 o)] per pair
                zall = {}
                for cp in (q, q + 1):
                    zts = [zpool.tile([TS, BO], F32R, tag="z", name=f"z{cp}_{p}")
                           for p in range(NP)]
                    for pp in range(NP // 2):
                        pss = []
                        for b in range(B):
                            ps = ps1.tile([TS, BO], F32, tag="s1")
                            for ic in range(2):
                                nc.tensor.matmul(
                                    ps[:],
                                    xs[cp][:, (b * 2 + ic) * TS:(b * 2 + ic + 1) * TS],
                                    ms[pp][ic][:],
                                    start=(ic == 0),
                                    stop=(ic == 1),
                                )
                            pss.append(ps)
                        for h in range(2):
                            z = zts[2 * pp + h]
                            for b in range(B):
                                nc.vector.tensor_copy(
                                    z[:, b * O:(b + 1) * O],
                                    pss[b][:, h * O:(h + 1) * O],
                                )
                    zall[cp] = zts
                # stage 2: contributions of both tiles to every c >= q,
                # fused into one PSUM accumulation group per c
                for c in range(q, CT):
                    cps = [cp for cp in (q, q + 1) if cp <= c]
                    yps = ps2.tile([TS, BO], F32, tag="s2")
                    n_mm = len(cps) * NP
                    i_mm = 0
                    for cp in cps:
                        for p in range(NP):
                            nc.tensor.matmul(
                                yps[:],
                                tbs[c - cp][:, p * TS:(p + 1) * TS],
                                zall[cp][p][:],
                                start=(i_mm == 0),
                                stop=(i_mm == n_mm - 1),
                            )
                            i_mm += 1
                    if q == 0:
                        nc.vector.tensor_copy(y_sb[c][:], yps[:])
                    else:
                        nc.vector.tensor_add(y_sb[c][:], y_sb[c][:], yps[:])
                    if c <= q + 1:  # y_sb[c] complete once its own tile passed
                        nc.sync.dma_start(yp_d[c], y_sb[c][:])

    _prune_init_barrier(nc)
    _split_sync_waits(nc)
    return nc


# ---------------------------------------------------------------------------
# Host side: input staging, sharding, gather
# ---------------------------------------------------------------------------

def _build_toeplitz(phi_eff):
    """tb[d, t, l] = phi_eff[d*TS + l - t] (0 where the index is negative)."""
    pad = np.zeros(L + TS - 1, np.float32)
    pad[TS - 1:] = phi_eff
    d = np.arange(CT)[:, None, None]
    t = np.arange(TS)[None, :, None]
    l = np.arange(TS)[None, None, :]
    return pad[d * TS + l - t + TS - 1]


_last_in_maps = None  # stashed for external profiling harnesses


def _prepare(x, phi, M_phi_plus, M_phi_minus):
    """Host prep: build per-core in_maps (no device execution)."""
    x = np.asarray(x, np.float32)
    phi = np.asarray(phi, np.float32)
    Mp = np.asarray(M_phi_plus, np.float32)
    Mm = np.asarray(M_phi_minus, np.float32)

    # [cp, i, (b, ic, t)]: per-sequence-tile chunks of x^T, one DMA per cp
    xT = np.ascontiguousarray(
        x.reshape(B, CT, TS, 2, TS).transpose(1, 4, 0, 3, 2)
    ).reshape(CT, TS, B * 2 * TS)
    sgn = ((-1.0) ** np.arange(L)).astype(np.float32)

    m_all = np.empty((2 * K, 2, TS, O), np.float32)
    tb_all = np.empty((2 * K, CT, TS, TS), np.float32)
    for g in range(2 * K):
        k, s = g // 2, g % 2
        m_all[g] = (Mm if s else Mp)[k].reshape(2, TS, O)
        phi_eff = phi[:, k] * (sgn if s else 1.0)
        tb_all[g] = _build_toeplitz(phi_eff)
    # fuse pair-pairs into columns: [pp, ic, i, (g0 o | g1 o)] per core
    m_fused = np.concatenate(
        [m_all[0::2], m_all[1::2]], axis=3
    )  # [24, 2, TS, 2*O] where row j holds (g=2j | g=2j+1)

    nc = _build_nc()
    in_maps = []
    for core in range(N_CORES):
        gs = slice(core * NP, (core + 1) * NP)
        pps = slice(core * (NP // 2), (core + 1) * (NP // 2))
        # [d, t, (p, l)] diagonal-major Toeplitz blocks for this core
        tb_core = np.ascontiguousarray(
            tb_all[gs].transpose(1, 2, 0, 3)
        ).reshape(CT, TS, NP * TS)
        in_maps.append({
            "xT": xT,
            "m": np.ascontiguousarray(m_fused[pps]),
            "tb": tb_core,
        })
    _last_in_maps = in_maps
    return nc, in_maps


def _gather(results):
    y = np.zeros((CT, TS, B, O), np.float64)
    for core in range(N_CORES):
        y += results[core]["yp"].reshape(CT, TS, B, O)
    return np.ascontiguousarray(
        y.transpose(2, 0, 1, 3).reshape(B, L, O)
    ).astype(np.float32)


def kernel(x, phi, M_phi_plus, M_phi_minus):
    nc, in_maps = _prepare(x, phi, M_phi_plus, M_phi_minus)
    res = run_bass_kernel_spmd(nc, in_maps, list(range(N_CORES)))
    return _gather(res.results)


# revision 21
# speedup vs baseline: 1.0095x; 1.0030x over previous
"""MiniSTU Trainium2 kernel (8 NeuronCores, Bass/Tile).

Math: the reference's FFT convolution + einsum collapses to
    y[b,l,o] = sum_g sum_{t<=l} phi_eff_g[l-t] * (x[b,t] @ M_g)[o]
over g in the 48 (filter k, sign) pairs, where phi_eff carries the
(-1)^s alternation for the minus branch (the two sgn factors in the
reference combine to (-1)^(l-t), i.e. an alternating filter).

Device algorithm per core (6 pairs per core, filter-dim sharding):
  stage 1: Z_g[t, (b,o)] = xT_tile.T @ M_g       (PE, f32r)
  stage 2: y[c] += Toeplitz(phi_eff_g)[c-cp].T @ Z_g[cp]   (PE, f32r)
Toeplitz blocks are expanded on host from phi. The 8 per-core partial
outputs are summed on host (the gather for this sharding).
"""

import numpy as np

import concourse.bass as bass
import concourse.tile as tile
from concourse import mybir
from concourse.bass_utils import run_bass_kernel_spmd
from concourse.vector_clock import ScopedClock

L = 2048
K = 24
I = 256
O = 256
B = 2
TS = 128          # tile size along sequence
CT = L // TS      # 16 sequence tiles
NP = 6            # (k, sign) pairs per core
N_CORES = 8
BO = B * O        # 512 fused (b, o) columns
F32 = mybir.dt.float32
F32R = mybir.dt.float32r


# ---------------------------------------------------------------------------
# Workarounds for this container's walrus: it rejects any instruction that
# carries more than one sync-wait command.
# ---------------------------------------------------------------------------

def _prune_init_barrier(nc):
    """Drop the Bass-init all-engine EVSEM barrier and the unused const
    memsets from the 'main' bb (~3us of EVSEM latency before any work).
    Register init is per-engine; Tile emits its own sems for every
    cross-engine dependency, so the startup barrier guards nothing here."""
    for f in nc.m.functions:
        for blk in f.blocks:
            if blk.name != "main":
                continue
            keep = []
            for inst in blk.instructions:
                nm = type(inst).__name__
                if nm in ("InstMemset", "InstDrain", "InstEventSemaphore"):
                    continue
                keep.append(inst)
            blk.instructions = keep


def _split_sync_waits(nc, max_waits=1):
    """Hoist extra sem-waits onto same-engine NOPs inserted right before the
    offending instruction; queue order keeps the semantics identical."""
    for f in nc.m.functions:
        for blk in f.blocks:
            insts = list(blk.instructions)
            out = []
            changed = False
            for inst in insts:
                si = getattr(inst, "sync_info", None)
                waits = list(si.on_wait) if si is not None else []
                if len(waits) > max_waits:
                    changed = True
                    extra, keep = waits[:-max_waits], waits[-max_waits:]
                    for j in range(0, len(extra), max_waits):
                        nop = mybir.InstNoOp(
                            name=nc.get_next_instruction_name(), ins=[], outs=[]
                        )
                        nop.engine = inst.engine
                        nop.sync_info = mybir.SyncInfo(
                            on_wait=extra[j : j + max_waits], on_update=[]
                        )
                        out.append(nop)
                    inst.sync_info = mybir.SyncInfo(
                        on_wait=keep, on_update=list(si.on_update)
                    )
                out.append(inst)
            if changed:
                blk.instructions = out


class _TC(tile.TileContext):
    """TileContext whose tail drain spreads its waits over 1-wait NOPs."""

    def _drain_and_barrier(self, tick_clock, wait_clock):
        nc = self.nc
        nop_inst = nc.sync.nop()
        wait_clock.add_sem_waits(
            nop_inst.ins, ScopedClock({None: tick_clock.global_clock})
        )
        si = nop_inst.ins.sync_info
        if si is not None and len(si.on_wait) > 1:
            waits = list(si.on_wait)
            nop_inst.ins.sync_info = mybir.SyncInfo(
                on_wait=waits[:1], on_update=list(si.on_update)
            )
            for w in waits[1:]:
                extra = nc.sync.nop().ins
                extra.sync_info = mybir.SyncInfo(on_wait=[w], on_update=[])
        nc.sync.drain()
        # No tail barriers / sem clearing: nothing runs after this kernel,
        # and all output DMAs are issued (and drained) on the SP queue.
        assert self.sems is not None
        popped = nc._tile_sem_poison_stack.pop()
        assert popped is self._sem_poison


# ---------------------------------------------------------------------------
# Device program (identical on all 8 cores; per-core data differs)
# ---------------------------------------------------------------------------

def _build_nc():
    nc = bass.Bass("TRN2", target_bir_lowering=False, debug=False,
                   num_devices=N_CORES)
    # x batched per sequence tile: [cp, i, (b, ic, t)]
    xT_d = nc.dram_tensor("xT", [CT, TS, B * 2 * TS], F32R, kind="ExternalInput")
    # M fused per pair-pair: [pp, ic, i, (p0 o | p1 o)]
    m_d = nc.dram_tensor("m", [NP // 2, 2, TS, 2 * O], F32R, kind="ExternalInput")
    # Toeplitz blocks batched per diagonal: [d, t, (p, l)]
    tb_d = nc.dram_tensor("tb", [CT, TS, NP * TS], F32R, kind="ExternalInput")
    yp_d = nc.dram_tensor("yp", [CT, TS, BO], F32, kind="ExternalOutput")

    with _TC(nc) as tc:
        with (
            tc.tile_pool(name="const", bufs=1) as cpool,
            tc.tile_pool(name="ys", bufs=1) as ypool,
            tc.tile_pool(name="z", bufs=18) as zpool,
            tc.tile_pool(name="ps1", bufs=4, space="PSUM") as ps1,
            tc.tile_pool(name="ps2", bufs=4, space="PSUM") as ps2,
        ):
            ms = [[cpool.tile([TS, 2 * O], F32R, tag=f"m{pp}{ic}", name=f"m{pp}{ic}")
                   for ic in range(2)] for pp in range(NP // 2)]
            xs = [cpool.tile([TS, B * 2 * TS], F32R, tag=f"x{cp}", name=f"x{cp}")
                  for cp in range(CT)]
            tbs = [cpool.tile([TS, NP * TS], F32R, tag=f"t{d}", name=f"t{d}")
                   for d in range(CT)]
            # strict consumption order; the critical head transfers
            # (ms[0][*], x0) are spread over four different DMA queues so
            # the first stage-1 group is not gated on one ring's bandwidth
            nc.sync.dma_start(ms[0][0][:, :O], m_d[0, 0, :, :O])
            nc.gpsimd.dma_start(ms[0][1][:, :O], m_d[0, 1, :, :O])
            nc.scalar.dma_start(xs[0][:], xT_d[0])
            nc.sync.dma_start(ms[0][0][:, O:], m_d[0, 0, :, O:])
            nc.gpsimd.dma_start(ms[0][1][:, O:], m_d[0, 1, :, O:])
            nc.sync.dma_start(ms[1][0][:], m_d[1, 0])
            nc.gpsimd.dma_start(ms[1][1][:], m_d[1, 1])
            nc.scalar.dma_start(ms[2][0][:], m_d[2, 0])
            nc.sync.dma_start(ms[2][1][:], m_d[2, 1])
            nc.gpsimd.dma_start(xs[1][:], xT_d[1])
            nc.sync.dma_start(tbs[0][:], tb_d[0])
            for s in range(1, CT):
                if s + 1 < CT:
                    nc.gpsimd.dma_start(xs[s + 1][:], xT_d[s + 1])
                nc.sync.dma_start(tbs[s][:], tb_d[s])


            y_sb = [ypool.tile([TS, BO], F32, tag=f"y{c}", name=f"ysb{c}") for c in range(CT)]

            for q in range(0, CT, 2):
                # stage 1 for cp = q and q+1: Z[t, (b0 o | b1 o)] per pair
                zall = {}
                for cp in (q, q + 1):
                    zts = [zpool.tile([TS, BO], F32R, tag="z", name=f"z{cp}_{p}")
                           for p in range(NP)]
                    for pp in range(NP // 2):
                        pss = []
                        for b in range(B):
                            ps = ps1.tile([TS, BO], F32, tag="s1")
                            for ic in range(2):
                                nc.tensor.matmul(
                                    ps[:],
                                    xs[cp][:, (b * 2 + ic) * TS:(b * 2 + ic + 1) * TS],
                                    ms[pp][ic][:],
                                    start=(ic == 0),
                                    stop=(ic == 1),
                                )
                            pss.append(ps)
                        for h in range(2):
                            z = zts[2 * pp + h]
                            for b in range(B):
                                nc.vector.tensor_copy(
                                    z[:, b * O:(b + 1) * O],
                                    pss[b][:, h * O:(h + 1) * O],
                                )
                    zall[cp] = zts
                # stage 2: contributions of both tiles to every c >= q,
                # fused into one PSUM accumulation group per c
                for c in range(q, CT):
                    cps = [cp for cp in (q, q + 1) if cp <= c]
                    yps = ps2.tile([TS, BO], F32, tag="s2")
                    n_mm = len(cps) * NP
                    i_mm = 0
                    for cp in cps:
                        for p in range(NP):
                            nc.tensor.matmul(
                                yps[:],
                                tbs[c - cp][:, p * TS:(p + 1) * TS],
                                zall[cp][p][:],
                                start=(i_mm == 0),
                                stop=(i_mm == n_mm - 1),
                            )
                            i_mm += 1
                    if q == 0:
                        nc.vector.tensor_copy(y_sb[c][:], yps[:])
                    else:
                        nc.vector.tensor_add(y_sb[c][:], y_sb[c][:], yps[:])
                    if c <= q + 1:  # y_sb[c] complete once its own tile passed
                        nc.sync.dma_start(yp_d[c], y_sb[c][:])

    _prune_init_barrier(nc)
    _split_sync_waits(nc)
    return nc


# ---------------------------------------------------------------------------
# Host side: input staging, sharding, gather
# ---------------------------------------------------------------------------

def _build_toeplitz(phi_eff):
    """tb[d, t, l] = phi_eff[d*TS + l - t] (0 where the index is negative)."""
    pad = np.zeros(L + TS - 1, np.float32)
    pad[TS - 1:] = phi_eff
    d = np.arange(CT)[:, None, None]
    t = np.arange(TS)[None, :, None]
    l = np.arange(TS)[None, None, :]
    return pad[d * TS + l - t + TS - 1]


_last_in_maps = None  # stashed for external profiling harnesses


def _prepare(x, phi, M_phi_plus, M_phi_minus):
    """Host prep: build per-core in_maps (no device execution)."""
    x = np.asarray(x, np.float32)
    phi = np.asarray(phi, np.float32)
    Mp = np.asarray(M_phi_plus, np.float32)
    Mm = np.asarray(M_phi_minus, np.float32)

    # [cp, i, (b, ic, t)]: per-sequence-tile chunks of x^T, one DMA per cp
    xT = np.ascontiguousarray(
        x.reshape(B, CT, TS, 2, TS).transpose(1, 4, 0, 3, 2)
    ).reshape(CT, TS, B * 2 * TS)
    sgn = ((-1.0) ** np.arange(L)).astype(np.float32)

    m_all = np.empty((2 * K, 2, TS, O), np.float32)
    tb_all = np.empty((2 * K, CT, TS, TS), np.float32)
    for g in range(2 * K):
        k, s = g // 2, g % 2
        m_all[g] = (Mm if s else Mp)[k].reshape(2, TS, O)
        phi_eff = phi[:, k] * (sgn if s else 1.0)
        tb_all[g] = _build_toeplitz(phi_eff)
    # fuse pair-pairs into columns: [pp, ic, i, (g0 o | g1 o)] per core
    m_fused = np.concatenate(
        [m_all[0::2], m_all[1::2]], axis=3
    )  # [24, 2, TS, 2*O] where row j holds (g=2j | g=2j+1)

    nc = _build_nc()
    in_maps = []
    for core in range(N_CORES):
        gs = slice(core * NP, (core + 1) * NP)
        pps = slice(core * (NP // 2), (core + 1) * (NP // 2))
        # [d, t, (p, l)] diagonal-major Toeplitz blocks for this core
        tb_core = np.ascontiguousarray(
            tb_all[gs].transpose(1, 2, 0, 3)
        ).reshape(CT, TS, NP * TS)
        in_maps.append({
            "xT": xT,
            "m": np.ascontiguousarray(m_fused[pps]),
            "tb": tb_core,
        })
    _last_in_maps = in_maps
    return nc, in_maps


def _gather(results):
    y = np.zeros((CT, TS, B, O), np.float64)
    for core in range(N_CORES):
        y += results[core]["yp"].reshape(CT, TS, B, O)
    return np.ascontiguousarray(
        y.transpose(2, 0, 1, 3).reshape(B, L, O)
    ).astype(np.float32)


def kernel(x, phi, M_phi_plus, M_phi_minus):
    nc, in_maps = _prepare(x, phi, M_phi_plus, M_phi_minus)
    res = run_bass_kernel_spmd(nc, in_maps, list(range(N_CORES)))
    return _gather(res.results)


# revision 22
# speedup vs baseline: 1.0112x; 1.0017x over previous
"""MiniSTU Trainium2 kernel (8 NeuronCores, Bass/Tile).

Math: the reference's FFT convolution + einsum collapses to
    y[b,l,o] = sum_g sum_{t<=l} phi_eff_g[l-t] * (x[b,t] @ M_g)[o]
over g in the 48 (filter k, sign) pairs, where phi_eff carries the
(-1)^s alternation for the minus branch (the two sgn factors in the
reference combine to (-1)^(l-t), i.e. an alternating filter).

Device algorithm per core (6 pairs per core, filter-dim sharding):
  stage 1: Z_g[t, (b,o)] = xT_tile.T @ M_g       (PE, f32r)
  stage 2: y[c] += Toeplitz(phi_eff_g)[c-cp].T @ Z_g[cp]   (PE, f32r)
Toeplitz blocks are expanded on host from phi. The 8 per-core partial
outputs are summed on host (the gather for this sharding).
"""

import numpy as np

import concourse.bass as bass
import concourse.tile as tile
from concourse import mybir
from concourse.bass_utils import run_bass_kernel_spmd
from concourse.vector_clock import ScopedClock

L = 2048
K = 24
I = 256
O = 256
B = 2
TS = 128          # tile size along sequence
CT = L // TS      # 16 sequence tiles
NP = 6            # (k, sign) pairs per core
N_CORES = 8
BO = B * O        # 512 fused (b, o) columns
F32 = mybir.dt.float32
F32R = mybir.dt.float32r


# ---------------------------------------------------------------------------
# Workarounds for this container's walrus: it rejects any instruction that
# carries more than one sync-wait command.
# ---------------------------------------------------------------------------

def _prune_init_barrier(nc):
    """Drop the Bass-init all-engine EVSEM barrier and the unused const
    memsets from the 'main' bb (~3us of EVSEM latency before any work).
    Register init is per-engine; Tile emits its own sems for every
    cross-engine dependency, so the startup barrier guards nothing here."""
    for f in nc.m.functions:
        for blk in f.blocks:
            if blk.name != "main":
                continue
            keep = []
            for inst in blk.instructions:
                nm = type(inst).__name__
                if nm in ("InstMemset", "InstDrain", "InstEventSemaphore"):
                    continue
                keep.append(inst)
            blk.instructions = keep


def _split_sync_waits(nc, max_waits=1):
    """Hoist extra sem-waits onto same-engine NOPs inserted right before the
    offending instruction; queue order keeps the semantics identical."""
    for f in nc.m.functions:
        for blk in f.blocks:
            insts = list(blk.instructions)
            out = []
            changed = False
            for inst in insts:
                si = getattr(inst, "sync_info", None)
                waits = list(si.on_wait) if si is not None else []
                if len(waits) > max_waits:
                    changed = True
                    extra, keep = waits[:-max_waits], waits[-max_waits:]
                    for j in range(0, len(extra), max_waits):
                        nop = mybir.InstNoOp(
                            name=nc.get_next_instruction_name(), ins=[], outs=[]
                        )
                        nop.engine = inst.engine
                        nop.sync_info = mybir.SyncInfo(
                            on_wait=extra[j : j + max_waits], on_update=[]
                        )
                        out.append(nop)
                    inst.sync_info = mybir.SyncInfo(
                        on_wait=keep, on_update=list(si.on_update)
                    )
                out.append(inst)
            if changed:
                blk.instructions = out


class _TC(tile.TileContext):
    """TileContext whose tail drain spreads its waits over 1-wait NOPs."""

    def _drain_and_barrier(self, tick_clock, wait_clock):
        nc = self.nc
        # No global-clock waits, tail barriers, or sem clearing: every DMA's
        # completion is awaited by its consumer, the output DMAs precede this
        # drain on the same SP queue, and nothing runs after this kernel.
        nc.sync.drain()
        assert self.sems is not None
        popped = nc._tile_sem_poison_stack.pop()
        assert popped is self._sem_poison


# ---------------------------------------------------------------------------
# Device program (identical on all 8 cores; per-core data differs)
# ---------------------------------------------------------------------------

def _build_nc():
    nc = bass.Bass("TRN2", target_bir_lowering=False, debug=False,
                   num_devices=N_CORES)
    # x batched per sequence tile: [cp, i, (b, ic, t)]
    xT_d = nc.dram_tensor("xT", [CT, TS, B * 2 * TS], F32R, kind="ExternalInput")
    # M fused per pair-pair: [pp, ic, i, (p0 o | p1 o)]
    m_d = nc.dram_tensor("m", [NP // 2, 2, TS, 2 * O], F32R, kind="ExternalInput")
    # Toeplitz blocks batched per diagonal: [d, t, (p, l)]
    tb_d = nc.dram_tensor("tb", [CT, TS, NP * TS], F32R, kind="ExternalInput")
    yp_d = nc.dram_tensor("yp", [CT, TS, BO], F32, kind="ExternalOutput")

    with _TC(nc) as tc:
        with (
            tc.tile_pool(name="const", bufs=1) as cpool,
            tc.tile_pool(name="ys", bufs=1) as ypool,
            tc.tile_pool(name="z", bufs=18) as zpool,
            tc.tile_pool(name="ps1", bufs=4, space="PSUM") as ps1,
            tc.tile_pool(name="ps2", bufs=4, space="PSUM") as ps2,
        ):
            ms = [[cpool.tile([TS, 2 * O], F32R, tag=f"m{pp}{ic}", name=f"m{pp}{ic}")
                   for ic in range(2)] for pp in range(NP // 2)]
            xs = [cpool.tile([TS, B * 2 * TS], F32R, tag=f"x{cp}", name=f"x{cp}")
                  for cp in range(CT)]
            tbs = [cpool.tile([TS, NP * TS], F32R, tag=f"t{d}", name=f"t{d}")
                   for d in range(CT)]
            # strict consumption order; the critical head transfers
            # (ms[0][*], x0) are spread over four different DMA queues so
            # the first stage-1 group is not gated on one ring's bandwidth
            nc.sync.dma_start(ms[0][0][:, :O], m_d[0, 0, :, :O])
            nc.gpsimd.dma_start(ms[0][1][:, :O], m_d[0, 1, :, :O])
            nc.scalar.dma_start(xs[0][:], xT_d[0])
            nc.sync.dma_start(ms[0][0][:, O:], m_d[0, 0, :, O:])
            nc.gpsimd.dma_start(ms[0][1][:, O:], m_d[0, 1, :, O:])
            nc.sync.dma_start(ms[1][0][:], m_d[1, 0])
            nc.gpsimd.dma_start(ms[1][1][:], m_d[1, 1])
            nc.scalar.dma_start(ms[2][0][:], m_d[2, 0])
            nc.sync.dma_start(ms[2][1][:], m_d[2, 1])
            nc.gpsimd.dma_start(xs[1][:], xT_d[1])
            nc.sync.dma_start(tbs[0][:], tb_d[0])
            for s in range(1, CT):
                if s + 1 < CT:
                    nc.gpsimd.dma_start(xs[s + 1][:], xT_d[s + 1])
                nc.sync.dma_start(tbs[s][:], tb_d[s])


            y_sb = [ypool.tile([TS, BO], F32, tag=f"y{c}", name=f"ysb{c}") for c in range(CT)]

            for q in range(0, CT, 2):
                # stage 1 for cp = q and q+1: Z[t, (b0 o | b1 o)] per pair
                zall = {}
                for cp in (q, q + 1):
                    zts = [zpool.tile([TS, BO], F32R, tag="z", name=f"z{cp}_{p}")
                           for p in range(NP)]
                    for pp in range(NP // 2):
                        pss = []
                        for b in range(B):
                            ps = ps1.tile([TS, BO], F32, tag="s1")
                            for ic in range(2):
                                nc.tensor.matmul(
                                    ps[:],
                                    xs[cp][:, (b * 2 + ic) * TS:(b * 2 + ic + 1) * TS],
                                    ms[pp][ic][:],
                                    start=(ic == 0),
                                    stop=(ic == 1),
                                )
                            pss.append(ps)
                        for h in range(2):
                            z = zts[2 * pp + h]
                            for b in range(B):
                                nc.vector.tensor_copy(
                                    z[:, b * O:(b + 1) * O],
                                    pss[b][:, h * O:(h + 1) * O],
                                )
                    zall[cp] = zts
                # stage 2: contributions of both tiles to every c >= q,
                # fused into one PSUM accumulation group per c
                for c in range(q, CT):
                    cps = [cp for cp in (q, q + 1) if cp <= c]
                    yps = ps2.tile([TS, BO], F32, tag="s2")
                    n_mm = len(cps) * NP
                    i_mm = 0
                    for cp in cps:
                        for p in range(NP):
                            nc.tensor.matmul(
                                yps[:],
                                tbs[c - cp][:, p * TS:(p + 1) * TS],
                                zall[cp][p][:],
                                start=(i_mm == 0),
                                stop=(i_mm == n_mm - 1),
                            )
                            i_mm += 1
                    if q == 0:
                        nc.vector.tensor_copy(y_sb[c][:], yps[:])
                    else:
                        nc.vector.tensor_add(y_sb[c][:], y_sb[c][:], yps[:])
                    if c <= q + 1:  # y_sb[c] complete once its own tile passed
                        nc.sync.dma_start(yp_d[c], y_sb[c][:])

    _prune_init_barrier(nc)
    _split_sync_waits(nc)
    return nc


# ---------------------------------------------------------------------------
# Host side: input staging, sharding, gather
# ---------------------------------------------------------------------------

def _build_toeplitz(phi_eff):
    """tb[d, t, l] = phi_eff[d*TS + l - t] (0 where the index is negative)."""
    pad = np.zeros(L + TS - 1, np.float32)
    pad[TS - 1:] = phi_eff
    d = np.arange(CT)[:, None, None]
    t = np.arange(TS)[None, :, None]
    l = np.arange(TS)[None, None, :]
    return pad[d * TS + l - t + TS - 1]


_last_in_maps = None  # stashed for external profiling harnesses


def _prepare(x, phi, M_phi_plus, M_phi_minus):
    """Host prep: build per-core in_maps (no device execution)."""
    x = np.asarray(x, np.float32)
    phi = np.asarray(phi, np.float32)
    Mp = np.asarray(M_phi_plus, np.float32)
    Mm = np.asarray(M_phi_minus, np.float32)

    # [cp, i, (b, ic, t)]: per-sequence-tile chunks of x^T, one DMA per cp
    xT = np.ascontiguousarray(
        x.reshape(B, CT, TS, 2, TS).transpose(1, 4, 0, 3, 2)
    ).reshape(CT, TS, B * 2 * TS)
    sgn = ((-1.0) ** np.arange(L)).astype(np.float32)

    m_all = np.empty((2 * K, 2, TS, O), np.float32)
    tb_all = np.empty((2 * K, CT, TS, TS), np.float32)
    for g in range(2 * K):
        k, s = g // 2, g % 2
        m_all[g] = (Mm if s else Mp)[k].reshape(2, TS, O)
        phi_eff = phi[:, k] * (sgn if s else 1.0)
        tb_all[g] = _build_toeplitz(phi_eff)
    # fuse pair-pairs into columns: [pp, ic, i, (g0 o | g1 o)] per core
    m_fused = np.concatenate(
        [m_all[0::2], m_all[1::2]], axis=3
    )  # [24, 2, TS, 2*O] where row j holds (g=2j | g=2j+1)

    nc = _build_nc()
    in_maps = []
    for core in range(N_CORES):
        gs = slice(core * NP, (core + 1) * NP)
        pps = slice(core * (NP // 2), (core + 1) * (NP // 2))
        # [d, t, (p, l)] diagonal-major Toeplitz blocks for this core
        tb_core = np.ascontiguousarray(
            tb_all[gs].transpose(1, 2, 0, 3)
        ).reshape(CT, TS, NP * TS)
        in_maps.append({
            "xT": xT,
            "m": np.ascontiguousarray(m_fused[pps]),
            "tb": tb_core,
        })
    _last_in_maps = in_maps
    return nc, in_maps


def _gather(results):
    y = np.zeros((CT, TS, B, O), np.float64)
    for core in range(N_CORES):
        y += results[core]["yp"].reshape(CT, TS, B, O)
    return np.ascontiguousarray(
        y.transpose(2, 0, 1, 3).reshape(B, L, O)
    ).astype(np.float32)


def kernel(x, phi, M_phi_plus, M_phi_minus):
    nc, in_maps = _prepare(x, phi, M_phi_plus, M_phi_minus)
    res = run_bass_kernel_spmd(nc, in_maps, list(range(N_CORES)))
    return _gather(res.results)


# revision 27
# speedup vs baseline: 1.0149x; 1.0037x over previous
"""MiniSTU Trainium2 kernel (8 NeuronCores, Bass/Tile).

Math: the reference's FFT convolution + einsum collapses to
    y[b,l,o] = sum_g sum_{t<=l} phi_eff_g[l-t] * (x[b,t] @ M_g)[o]
over g in the 48 (filter k, sign) pairs, where phi_eff carries the
(-1)^s alternation for the minus branch (the two sgn factors in the
reference combine to (-1)^(l-t), i.e. an alternating filter).

Device algorithm per core (6 pairs per core, filter-dim sharding):
  stage 1: Z_g[t, (b,o)] = xT_tile.T @ M_g       (PE, f32r)
  stage 2: y[c] += Toeplitz(phi_eff_g)[c-cp].T @ Z_g[cp]   (PE, f32r)
Toeplitz blocks are expanded on host from phi. The 8 per-core partial
outputs are summed on host (the gather for this sharding).
"""

import numpy as np

import concourse.bass as bass
import concourse.tile as tile
from concourse import mybir
from concourse.bass_utils import run_bass_kernel_spmd
from concourse.vector_clock import ScopedClock

L = 2048
K = 24
I = 256
O = 256
B = 2
TS = 128          # tile size along sequence
CT = L // TS      # 16 sequence tiles
NP = 6            # (k, sign) pairs per core
N_CORES = 8
BO = B * O        # 512 fused (b, o) columns
F32 = mybir.dt.float32
F32R = mybir.dt.float32r


# ---------------------------------------------------------------------------
# Workarounds for this container's walrus: it rejects any instruction that
# carries more than one sync-wait command.
# ---------------------------------------------------------------------------

def _prune_init_barrier(nc):
    """Drop the Bass-init all-engine EVSEM barrier and the unused const
    memsets from the 'main' bb (~3us of EVSEM latency before any work).
    Register init is per-engine; Tile emits its own sems for every
    cross-engine dependency, so the startup barrier guards nothing here."""
    for f in nc.m.functions:
        for blk in f.blocks:
            if blk.name != "main":
                continue
            keep = []
            for inst in blk.instructions:
                nm = type(inst).__name__
                if nm in ("InstMemset", "InstDrain", "InstEventSemaphore"):
                    continue
                keep.append(inst)
            blk.instructions = keep


def _split_sync_waits(nc, max_waits=1):
    """Hoist extra sem-waits onto same-engine NOPs inserted right before the
    offending instruction; queue order keeps the semantics identical."""
    for f in nc.m.functions:
        for blk in f.blocks:
            insts = list(blk.instructions)
            out = []
            changed = False
            for inst in insts:
                si = getattr(inst, "sync_info", None)
                waits = list(si.on_wait) if si is not None else []
                if len(waits) > max_waits:
                    changed = True
                    extra, keep = waits[:-max_waits], waits[-max_waits:]
                    for j in range(0, len(extra), max_waits):
                        nop = mybir.InstNoOp(
                            name=nc.get_next_instruction_name(), ins=[], outs=[]
                        )
                        nop.engine = inst.engine
                        nop.sync_info = mybir.SyncInfo(
                            on_wait=extra[j : j + max_waits], on_update=[]
                        )
                        out.append(nop)
                    inst.sync_info = mybir.SyncInfo(
                        on_wait=keep, on_update=list(si.on_update)
                    )
                out.append(inst)
            if changed:
                blk.instructions = out


class _TC(tile.TileContext):
    """TileContext whose tail drain spreads its waits over 1-wait NOPs."""

    def _drain_and_barrier(self, tick_clock, wait_clock):
        nc = self.nc
        # No global-clock waits, tail barriers, or sem clearing: every DMA's
        # completion is awaited by its consumer, the output DMAs precede this
        # drain on the same SP queue, and nothing runs after this kernel.
        nc.sync.drain()
        assert self.sems is not None
        popped = nc._tile_sem_poison_stack.pop()
        assert popped is self._sem_poison


# ---------------------------------------------------------------------------
# Device program (identical on all 8 cores; per-core data differs)
# ---------------------------------------------------------------------------

def _build_nc():
    nc = bass.Bass("TRN2", target_bir_lowering=False, debug=False,
                   num_devices=N_CORES)
    # x batched per sequence tile: [cp, i, (b, ic, t)]
    xT_d = nc.dram_tensor("xT", [CT, TS, B * 2 * TS], F32R, kind="ExternalInput")
    # M fused per pair-pair: [pp, ic, i, (p0 o | p1 o)]
    m_d = nc.dram_tensor("m", [NP // 2, 2, TS, 2 * O], F32R, kind="ExternalInput")
    # Toeplitz blocks batched per diagonal: [d, t, (p, l)]
    tb_d = nc.dram_tensor("tb", [CT, TS, NP * TS], F32R, kind="ExternalInput")
    yp_d = nc.dram_tensor("yp", [CT, TS, BO], F32, kind="ExternalOutput")

    with _TC(nc) as tc:
        with (
            tc.tile_pool(name="const", bufs=1) as cpool,
            tc.tile_pool(name="ys", bufs=1) as ypool,
            tc.tile_pool(name="z", bufs=18) as zpool,
            tc.tile_pool(name="ps1", bufs=4, space="PSUM") as ps1,
            tc.tile_pool(name="ps2", bufs=4, space="PSUM") as ps2,
        ):
            ms = [[cpool.tile([TS, 2 * O], F32R, tag=f"m{pp}{ic}", name=f"m{pp}{ic}")
                   for ic in range(2)] for pp in range(NP // 2)]
            xs = [cpool.tile([TS, B * 2 * TS], F32R, tag=f"x{cp}", name=f"x{cp}")
                  for cp in range(CT)]
            tbs = [cpool.tile([TS, NP * TS], F32R, tag=f"t{d}", name=f"t{d}")
                   for d in range(CT)]
            # strict consumption order; the critical head transfers
            # (ms[0][*], x0) are spread over four different DMA queues so
            # the first stage-1 group is not gated on one ring's bandwidth
            nc.sync.dma_start(ms[0][0][:, :O], m_d[0, 0, :, :O])
            nc.gpsimd.dma_start(ms[0][1][:, :O], m_d[0, 1, :, :O])
            nc.scalar.dma_start(xs[0][:], xT_d[0])
            nc.sync.dma_start(ms[0][0][:, O:], m_d[0, 0, :, O:])
            nc.gpsimd.dma_start(ms[0][1][:, O:], m_d[0, 1, :, O:])
            nc.sync.dma_start(ms[1][0][:], m_d[1, 0])
            nc.gpsimd.dma_start(ms[1][1][:], m_d[1, 1])
            nc.scalar.dma_start(ms[2][0][:], m_d[2, 0])
            nc.sync.dma_start(ms[2][1][:], m_d[2, 1])
            nc.gpsimd.dma_start(xs[1][:], xT_d[1])
            nc.sync.dma_start(tbs[0][:], tb_d[0])
            for s in range(1, CT):
                if s + 1 < CT:
                    nc.gpsimd.dma_start(xs[s + 1][:], xT_d[s + 1])
                nc.sync.dma_start(tbs[s][:], tb_d[s])


            y_sb = [ypool.tile([TS, BO], F32, tag=f"y{c}", name=f"ysb{c}") for c in range(CT)]

            for q in range(0, CT, 2):
                # stage 1 for cp = q and q+1: Z[t, (b0 o | b1 o)] per pair
                zall = {}
                for cp in (q, q + 1):
                    zts = [zpool.tile([TS, BO], F32R, tag="z", name=f"z{cp}_{p}")
                           for p in range(NP)]
                    for pp in range(NP // 2):
                        pss = []
                        for b in range(B):
                            ps = ps1.tile([TS, BO], F32, tag="s1")
                            for ic in range(2):
                                nc.tensor.matmul(
                                    ps[:],
                                    xs[cp][:, (b * 2 + ic) * TS:(b * 2 + ic + 1) * TS],
                                    ms[pp][ic][:],
                                    start=(ic == 0),
                                    stop=(ic == 1),
                                )
                            pss.append(ps)
                        for h in range(2):
                            z = zts[2 * pp + h]
                            for b in range(B):
                                nc.vector.tensor_copy(
                                    z[:, b * O:(b + 1) * O],
                                    pss[b][:, h * O:(h + 1) * O],
                                )
                    zall[cp] = zts
                # stage 2: contributions of both tiles to every c >= q,
                # fused into one PSUM accumulation group per c
                for c in range(q, CT):
                    cps = [cp for cp in (q, q + 1) if cp <= c]
                    yps = ps2.tile([TS, BO], F32, tag="s2")
                    n_mm = len(cps) * NP
                    i_mm = 0
                    for cp in cps:
                        for p in range(NP):
                            nc.tensor.matmul(
                                yps[:],
                                tbs[c - cp][:, p * TS:(p + 1) * TS],
                                zall[cp][p][:],
                                start=(i_mm == 0),
                                stop=(i_mm == n_mm - 1),
                            )
                            i_mm += 1
                    if q == 0:
                        nc.vector.tensor_copy(y_sb[c][:], yps[:])
                    else:
                        nc.vector.tensor_add(y_sb[c][:], y_sb[c][:], yps[:])
                    if c <= q + 1:  # y_sb[c] complete once its own tile passed
                        nc.sync.dma_start(yp_d[c], y_sb[c][:])

    _prune_init_barrier(nc)
    _split_sync_waits(nc)
    return nc


# ---------------------------------------------------------------------------
# Host side: input staging, sharding, gather
# ---------------------------------------------------------------------------

def _build_toeplitz(phi_eff):
    """tb[d, t, l] = phi_eff[d*TS + l - t] (0 where the index is negative)."""
    pad = np.zeros(L + TS - 1, np.float32)
    pad[TS - 1:] = phi_eff
    d = np.arange(CT)[:, None, None]
    t = np.arange(TS)[None, :, None]
    l = np.arange(TS)[None, None, :]
    return pad[d * TS + l - t + TS - 1]


_last_in_maps = None  # stashed for external profiling harnesses


def _prepare(x, phi, M_phi_plus, M_phi_minus):
    """Host prep: build per-core in_maps (no device execution)."""
    x = np.asarray(x, np.float32)
    phi = np.asarray(phi, np.float32)
    Mp = np.asarray(M_phi_plus, np.float32)
    Mm = np.asarray(M_phi_minus, np.float32)

    # [cp, i, (b, ic, t)]: per-sequence-tile chunks of x^T, one DMA per cp
    xT = np.ascontiguousarray(
        x.reshape(B, CT, TS, 2, TS).transpose(1, 4, 0, 3, 2)
    ).reshape(CT, TS, B * 2 * TS)
    sgn = ((-1.0) ** np.arange(L)).astype(np.float32)

    m_all = np.empty((2 * K, 2, TS, O), np.float32)
    tb_all = np.empty((2 * K, CT, TS, TS), np.float32)
    for g in range(2 * K):
        k, s = g // 2, g % 2
        m_all[g] = (Mm if s else Mp)[k].reshape(2, TS, O)
        phi_eff = phi[:, k] * (sgn if s else 1.0)
        tb_all[g] = _build_toeplitz(phi_eff)
    # fuse pair-pairs into columns: [pp, ic, i, (g0 o | g1 o)] per core
    m_fused = np.concatenate(
        [m_all[0::2], m_all[1::2]], axis=3
    )  # [24, 2, TS, 2*O] where row j holds (g=2j | g=2j+1)

    nc = _build_nc()
    in_maps = []
    for core in range(N_CORES):
        gs = slice(core * NP, (core + 1) * NP)
        pps = slice(core * (NP // 2), (core + 1) * (NP // 2))
        # [d, t, (p, l)] diagonal-major Toeplitz blocks for this core
        tb_core = np.ascontiguousarray(
            tb_all[gs].transpose(1, 2, 0, 3)
        ).reshape(CT, TS, NP * TS)
        in_maps.append({
            "xT": xT,
            "m": np.ascontiguousarray(m_fused[pps]),
            "tb": tb_core,
        })
    _last_in_maps = in_maps
    return nc, in_maps


def _gather(results):
    y = np.zeros((CT, TS, B, O), np.float64)
    for core in range(N_CORES):
        y += results[core]["yp"].reshape(CT, TS, B, O)
    return np.ascontiguousarray(
        y.transpose(2, 0, 1, 3).reshape(B, L, O)
    ).astype(np.float32)


def kernel(x, phi, M_phi_plus, M_phi_minus):
    nc, in_maps = _prepare(x, phi, M_phi_plus, M_phi_minus)
    res = run_bass_kernel_spmd(nc, in_maps, list(range(N_CORES)))
    return _gather(res.results)


# revision 28
# speedup vs baseline: 1.0946x; 1.0786x over previous
"""MiniSTU Trainium2 kernel (8 NeuronCores, Bass/Tile).

Math: the reference's FFT convolution + einsum collapses to
    y[b,l,o] = sum_g sum_{t<=l} phi_eff_g[l-t] * (x[b,t] @ M_g)[o]
over g in the 48 (filter k, sign) pairs, where phi_eff carries the
(-1)^s alternation for the minus branch (the two sgn factors in the
reference combine to (-1)^(l-t), i.e. an alternating filter).

Device algorithm per core (6 pairs per core, filter-dim sharding):
  stage 1: Z_g[t, (b,o)] = xT_tile.T @ M_g       (PE, f32r)
  stage 2: y[c] += Toeplitz(phi_eff_g)[c-cp].T @ Z_g[cp]   (PE, f32r)
Toeplitz blocks are expanded on host from phi. The 8 per-core partial
outputs are summed on host (the gather for this sharding).
"""

import numpy as np

import concourse.bass as bass
import concourse.tile as tile
from concourse import mybir
from concourse.bass_utils import run_bass_kernel_spmd
from concourse.vector_clock import ScopedClock

L = 2048
K = 24
I = 256
O = 256
B = 2
TS = 128          # tile size along sequence
CT = L // TS      # 16 sequence tiles
NP = 6            # (k, sign) pairs per core
N_CORES = 8
BO = B * O        # 512 fused (b, o) columns
F32 = mybir.dt.float32
F32R = mybir.dt.float32r
BF16 = mybir.dt.bfloat16
RSH = 32           # shared far-field basis rank (full 32-row strips)


# ---------------------------------------------------------------------------
# Workarounds for this container's walrus: it rejects any instruction that
# carries more than one sync-wait command.
# ---------------------------------------------------------------------------

def _prune_init_barrier(nc):
    """Drop the Bass-init all-engine EVSEM barrier and the unused const
    memsets from the 'main' bb (~3us of EVSEM latency before any work).
    Register init is per-engine; Tile emits its own sems for every
    cross-engine dependency, so the startup barrier guards nothing here."""
    for f in nc.m.functions:
        for blk in f.blocks:
            if blk.name != "main":
                continue
            keep = []
            for inst in blk.instructions:
                nm = type(inst).__name__
                if nm in ("InstMemset", "InstDrain", "InstEventSemaphore"):
                    continue
                keep.append(inst)
            blk.instructions = keep


def _split_sync_waits(nc, max_waits=1):
    """Hoist extra sem-waits onto same-engine NOPs inserted right before the
    offending instruction; queue order keeps the semantics identical."""
    for f in nc.m.functions:
        for blk in f.blocks:
            insts = list(blk.instructions)
            out = []
            changed = False
            for inst in insts:
                si = getattr(inst, "sync_info", None)
                waits = list(si.on_wait) if si is not None else []
                if len(waits) > max_waits:
                    changed = True
                    extra, keep = waits[:-max_waits], waits[-max_waits:]
                    for j in range(0, len(extra), max_waits):
                        nop = mybir.InstNoOp(
                            name=nc.get_next_instruction_name(), ins=[], outs=[]
                        )
                        nop.engine = inst.engine
                        nop.sync_info = mybir.SyncInfo(
                            on_wait=extra[j : j + max_waits], on_update=[]
                        )
                        out.append(nop)
                    inst.sync_info = mybir.SyncInfo(
                        on_wait=keep, on_update=list(si.on_update)
                    )
                out.append(inst)
            if changed:
                blk.instructions = out


class _TC(tile.TileContext):
    """TileContext whose tail drain spreads its waits over 1-wait NOPs."""

    def _drain_and_barrier(self, tick_clock, wait_clock):
        nc = self.nc
        # No global-clock waits, tail barriers, or sem clearing: every DMA's
        # completion is awaited by its consumer, the output DMAs precede this
        # drain on the same SP queue, and nothing runs after this kernel.
        nc.sync.drain()
        assert self.sems is not None
        popped = nc._tile_sem_poison_stack.pop()
        assert popped is self._sem_poison


# ---------------------------------------------------------------------------
# Device program (identical on all 8 cores; per-core data differs)
# ---------------------------------------------------------------------------

def _build_nc():
    nc = bass.Bass("TRN2", target_bir_lowering=False, debug=False,
                   num_devices=N_CORES)
    # x batched per sequence tile: [cp, i, (b, ic, t)]
    xT_d = nc.dram_tensor("xT", [CT, TS, B * 2 * TS], F32R, kind="ExternalInput")
    # M fused per pair-pair: [pp, ic, i, (p0 o | p1 o)]
    m_d = nc.dram_tensor("m", [NP // 2, 2, TS, 2 * O], F32R, kind="ExternalInput")
    # dense Toeplitz blocks, diagonals 0..1 only: [d, t, (p, l)]
    tb_d = nc.dram_tensor("tb", [2, TS, NP * TS], F32R, kind="ExternalInput")
    # shared far-field t-basis [t, R]; G stacks [d, bank, (3 pairs x R), l]
    pb_d = nc.dram_tensor("pb", [TS, RSH], F32R, kind="ExternalInput")
    gf_d = nc.dram_tensor("gf", [CT - 2, 2, 3 * RSH, TS], BF16,
                          kind="ExternalInput")
    yp_d = nc.dram_tensor("yp", [CT, TS, BO], F32, kind="ExternalOutput")

    with _TC(nc) as tc:
        with (
            tc.tile_pool(name="const", bufs=1) as cpool,
            tc.tile_pool(name="ys", bufs=1) as ypool,
            tc.tile_pool(name="z", bufs=18) as zpool,
            tc.tile_pool(name="ps1", bufs=3, space="PSUM") as ps1,
            tc.tile_pool(name="ps2", bufs=3, space="PSUM") as ps2,
            tc.tile_pool(name="psW", bufs=2, space="PSUM") as psW,
        ):
            ms = [[cpool.tile([TS, 2 * O], F32R, tag=f"m{pp}{ic}", name=f"m{pp}{ic}")
                   for ic in range(2)] for pp in range(NP // 2)]
            xs = [cpool.tile([TS, B * 2 * TS], F32R, tag=f"x{cp}", name=f"x{cp}")
                  for cp in range(CT)]
            tbs = [cpool.tile([TS, NP * TS], F32R, tag=f"t{d}", name=f"t{d}")
                   for d in range(2)]
            pbt = cpool.tile([TS, RSH], F32R, tag="pb", name="pbt")
            gts = [[cpool.tile([3 * RSH, TS], BF16, tag=f"g{d}_{j}",
                               name=f"g{d}_{j}") for j in range(2)]
                   for d in range(CT - 2)]
            wts = [[cpool.tile([3 * RSH, BO], BF16, tag=f"w{cp}_{j}",
                               name=f"w{cp}_{j}") for j in range(2)]
                   for cp in range(CT)]
            # strict consumption order; the critical head transfers
            # (ms[0][*], x0) are spread over four different DMA queues so
            # the first stage-1 group is not gated on one ring's bandwidth
            nc.sync.dma_start(ms[0][0][:, :O], m_d[0, 0, :, :O])
            nc.gpsimd.dma_start(ms[0][1][:, :O], m_d[0, 1, :, :O])
            nc.scalar.dma_start(xs[0][:], xT_d[0])
            nc.sync.dma_start(ms[0][0][:, O:], m_d[0, 0, :, O:])
            nc.gpsimd.dma_start(ms[0][1][:, O:], m_d[0, 1, :, O:])
            nc.sync.dma_start(ms[1][0][:], m_d[1, 0])
            nc.gpsimd.dma_start(ms[1][1][:], m_d[1, 1])
            nc.scalar.dma_start(ms[2][0][:], m_d[2, 0])
            nc.sync.dma_start(ms[2][1][:], m_d[2, 1])
            nc.gpsimd.dma_start(xs[1][:], xT_d[1])
            nc.sync.dma_start(tbs[0][:], tb_d[0])
            nc.sync.dma_start(tbs[1][:], tb_d[1])
            nc.sync.dma_start(pbt[:], pb_d[:])
            for s in range(1, CT):
                if s + 1 < CT:
                    nc.gpsimd.dma_start(xs[s + 1][:], xT_d[s + 1])
                if s - 1 < CT - 2:
                    nc.sync.dma_start(gts[s - 1][0][:], gf_d[s - 1, 0])
                    nc.sync.dma_start(gts[s - 1][1][:], gf_d[s - 1, 1])


            y_sb = [ypool.tile([TS, BO], F32, tag=f"y{c}", name=f"ysb{c}") for c in range(CT)]

            for q in range(0, CT, 2):
                # stage 1 for cp = q and q+1: Z[t, (b0 o | b1 o)] per pair
                zall = {}
                for cp in (q, q + 1):
                    zts = [zpool.tile([TS, BO], F32R, tag="z", name=f"z{cp}_{p}")
                           for p in range(NP)]
                    for pp in range(NP // 2):
                        pss = []
                        for b in range(B):
                            ps = ps1.tile([TS, BO], F32, tag="s1")
                            for ic in range(2):
                                nc.tensor.matmul(
                                    ps[:],
                                    xs[cp][:, (b * 2 + ic) * TS:(b * 2 + ic + 1) * TS],
                                    ms[pp][ic][:],
                                    start=(ic == 0),
                                    stop=(ic == 1),
                                )
                            pss.append(ps)
                        for h in range(2):
                            z = zts[2 * pp + h]
                            for b in range(B):
                                nc.vector.tensor_copy(
                                    z[:, b * O:(b + 1) * O],
                                    pss[b][:, h * O:(h + 1) * O],
                                )
                    zall[cp] = zts
                    # far-field projection W_p = P^T Z_p: each proj matmul
                    # outputs at PSUM base 0 (the only legal matmul dst
                    # base), then a DVE cast places it at its 32-aligned
                    # slot in the W stack.
                    for p in range(NP):
                        psw = psW.tile([RSH, BO], F32, tag="sW")
                        nc.tensor.matmul(psw[:], pbt[:], zts[p][:],
                                         start=True, stop=True)
                        nc.vector.tensor_copy(
                            wts[cp][p // 3][(p % 3) * RSH:(p % 3 + 1) * RSH, :],
                            psw[:],
                        )
                # stage 2: contributions of both tiles to every c >= q,
                # fused into one PSUM accumulation group per c
                for c in range(q, CT):
                    cps = [cp for cp in (q, q + 1) if cp <= c]
                    yps = ps2.tile([TS, BO], F32, tag="s2")
                    n_mm = sum(NP if c - cp <= 1 else 2 for cp in cps)
                    i_mm = 0
                    for cp in cps:
                        d = c - cp
                        if d <= 1:
                            for p in range(NP):
                                nc.tensor.matmul(
                                    yps[:],
                                    tbs[d][:, p * TS:(p + 1) * TS],
                                    zall[cp][p][:],
                                    start=(i_mm == 0),
                                    stop=(i_mm == n_mm - 1),
                                )
                                i_mm += 1
                        else:
                            for j in range(2):
                                nc.tensor.matmul(
                                    yps[:], gts[d - 2][j][:], wts[cp][j][:],
                                    start=(i_mm == 0),
                                    stop=(i_mm == n_mm - 1),
                                )
                                i_mm += 1
                    if q == 0:
                        nc.vector.tensor_copy(y_sb[c][:], yps[:])
                    else:
                        nc.vector.tensor_add(y_sb[c][:], y_sb[c][:], yps[:])
                    if c <= q + 1:  # y_sb[c] complete once its own tile passed
                        nc.sync.dma_start(yp_d[c], y_sb[c][:])

    _prune_init_barrier(nc)
    _split_sync_waits(nc)
    return nc


# ---------------------------------------------------------------------------
# Host side: input staging, sharding, gather
# ---------------------------------------------------------------------------

def _build_toeplitz(phi_eff):
    """tb[d, t, l] = phi_eff[d*TS + l - t] (0 where the index is negative)."""
    pad = np.zeros(L + TS - 1, np.float32)
    pad[TS - 1:] = phi_eff
    d = np.arange(CT)[:, None, None]
    t = np.arange(TS)[None, :, None]
    l = np.arange(TS)[None, None, :]
    return pad[d * TS + l - t + TS - 1]


_last_in_maps = None  # stashed for external profiling harnesses


def _prepare(x, phi, M_phi_plus, M_phi_minus):
    """Host prep: build per-core in_maps (no device execution)."""
    x = np.asarray(x, np.float32)
    phi = np.asarray(phi, np.float32)
    Mp = np.asarray(M_phi_plus, np.float32)
    Mm = np.asarray(M_phi_minus, np.float32)

    # [cp, i, (b, ic, t)]: per-sequence-tile chunks of x^T, one DMA per cp
    xT = np.ascontiguousarray(
        x.reshape(B, CT, TS, 2, TS).transpose(1, 4, 0, 3, 2)
    ).reshape(CT, TS, B * 2 * TS)
    sgn = ((-1.0) ** np.arange(L)).astype(np.float32)

    m_all = np.empty((2 * K, 2, TS, O), np.float32)
    tb_all = np.empty((2 * K, CT, TS, TS), np.float32)
    for g in range(2 * K):
        k, s = g // 2, g % 2
        m_all[g] = (Mm if s else Mp)[k].reshape(2, TS, O)
        phi_eff = phi[:, k] * (sgn if s else 1.0)
        tb_all[g] = _build_toeplitz(phi_eff)
    # shared far-field t-basis over all pairs' blocks (d >= 2)
    gram = np.zeros((TS, TS), np.float64)
    for g in range(2 * K):
        far = tb_all[g, 2:]
        gram += np.einsum('dtl,dsl->ts', far, far)
    _, evec = np.linalg.eigh(gram)
    Pb = np.ascontiguousarray(evec[:, ::-1][:, :RSH]).astype(np.float32)
    gf_all = np.einsum('tr,gdtl->gdrl', Pb, tb_all[:, 2:])  # [48,14,R,l]

    # fuse pair-pairs into columns: [pp, ic, i, (g0 o | g1 o)] per core
    m_fused = np.concatenate(
        [m_all[0::2], m_all[1::2]], axis=3
    )  # [24, 2, TS, 2*O] where row j holds (g=2j | g=2j+1)

    nc = _build_nc()
    in_maps = []
    for core in range(N_CORES):
        gs = slice(core * NP, (core + 1) * NP)
        pps = slice(core * (NP // 2), (core + 1) * (NP // 2))
        # dense diagonals 0..1: [d, t, (p, l)] for this core
        tb_core = np.ascontiguousarray(
            tb_all[gs, :2].transpose(1, 2, 0, 3)
        ).reshape(2, TS, NP * TS)
        # far G stacks: [d, bank j, 32*i+rho, l] in bf16
        import ml_dtypes
        gf_core = np.zeros((CT - 2, 2, 3 * RSH, TS), np.float32)
        for j in range(2):
            for i in range(3):
                gf_core[:, j, i * RSH:(i + 1) * RSH, :] = gf_all[gs][3 * j + i]
        in_maps.append({
            "xT": xT,
            "m": np.ascontiguousarray(m_fused[pps]),
            "tb": tb_core,
            "pb": Pb,
            "gf": gf_core.astype(ml_dtypes.bfloat16),
        })
    _last_in_maps = in_maps
    return nc, in_maps


def _gather(results):
    y = np.zeros((CT, TS, B, O), np.float64)
    for core in range(N_CORES):
        y += results[core]["yp"].reshape(CT, TS, B, O)
    return np.ascontiguousarray(
        y.transpose(2, 0, 1, 3).reshape(B, L, O)
    ).astype(np.float32)


def kernel(x, phi, M_phi_plus, M_phi_minus):
    nc, in_maps = _prepare(x, phi, M_phi_plus, M_phi_minus)
    res = run_bass_kernel_spmd(nc, in_maps, list(range(N_CORES)))
    return _gather(res.results)
